# revision 1
# baseline (speedup 1.0000x reference)
"""Multi-head causal self-attention (B=2, S=2048, D=2048, H=16) on 8 TRN2 cores.

Sharding: data parallel on batch (2) x tensor parallel on head groups (4 heads
per core). Each core computes QKV projections for its 512 q/k/v channels, the
causal attention for its 4 heads, and a partial output projection against its
512 columns of Wo. The host sums the 4 partials per batch and adds bo.

All matmul operands are fp16 (full PE rate, fast weight load, fp22 multiply
with fp32 accumulate); softmax statistics stay fp32. Scores are computed
directly in [k, q] orientation so the exp'd tiles feed the PV matmul as the
moving operand with no transposes; row sums come from an all-ones stationary
matmul (replicated across partitions) and normalization happens on the
PSUM->SBUF copy.
"""

import math
from contextlib import ExitStack

import numpy as np

import concourse.bass as bass
import concourse.tile as tile
from concourse import bacc, mybir
from concourse.bass_utils import run_bass_kernel_spmd

B, S, D, H, HD = 2, 2048, 2048, 16, 128
N_CORES = 8
HPC = 4          # heads per core
HJ = HPC * HD    # 512 projection channels per core
SG = 512         # column-group width for matmuls
ND = D // 128    # 16 contraction tiles over model dim
NS = S // 128    # 16 tiles over sequence
NG = S // SG     # 4 column groups over sequence

F32 = mybir.dt.float32
F16 = mybir.dt.float16
AX = mybir.AxisListType.X
ADD = mybir.AluOpType.add
MUL = mybir.AluOpType.mult
EXP = mybir.ActivationFunctionType.Exp

last_exec_time_ns = None


def _build():
    nc = bacc.Bacc("TRN2", target_bir_lowering=False, debug=False)

    xt = nc.dram_tensor("xt", [D, S], F16, kind="ExternalInput").ap()
    wq = nc.dram_tensor("wq", [D, HJ], F16, kind="ExternalInput").ap()
    wk = nc.dram_tensor("wk", [D, HJ], F16, kind="ExternalInput").ap()
    wv = nc.dram_tensor("wv", [D, HJ], F16, kind="ExternalInput").ap()
    wo = nc.dram_tensor("wo", [HJ, D], F16, kind="ExternalInput").ap()
    bq = nc.dram_tensor("bq", [HJ, 1], F32, kind="ExternalInput").ap()
    bk = nc.dram_tensor("bk", [HJ, 1], F32, kind="ExternalInput").ap()
    bv = nc.dram_tensor("bv", [1, HJ], F16, kind="ExternalInput").ap()
    mask = nc.dram_tensor("mask", [128, 128], F32, kind="ExternalInput").ap()
    ones = nc.dram_tensor("ones", [1, 128], F16, kind="ExternalInput").ap()
    out = nc.dram_tensor("out", [S, D], F32, kind="ExternalOutput").ap()

    with tile.TileContext(nc) as tc, ExitStack() as es:
        cpool = es.enter_context(tc.tile_pool(name="const", bufs=1))
        mask_sb = cpool.tile([128, 128], F32, name="mask", tag="mask")
        nc.sync.dma_start(mask_sb[:], mask[:])
        ones_sb = cpool.tile([1, 128], F16, name="ones", tag="ones")
        nc.sync.dma_start(ones_sb[:], ones[:])
        bv_sb = cpool.tile([1, HJ], F16, name="bv", tag="bv")
        nc.sync.dma_start(bv_sb[:], bv[:])
        onesm_sb = cpool.tile([128, 128], F16, name="onesm_sb", tag="onesm")
        nc.gpsimd.memset(onesm_sb[:], 1.0)
        bq_sb = []
        bk_sb = []
        for i in range(HPC):
            t = cpool.tile([128, 1], F32, name=f"bq{i}", tag=f"bq{i}")
            nc.sync.dma_start(t[:], bq[i * 128:(i + 1) * 128, :])
            bq_sb.append(t)
            t = cpool.tile([128, 1], F32, name=f"bk{i}", tag=f"bk{i}")
            nc.sync.dma_start(t[:], bk[i * 128:(i + 1) * 128, :])
            bk_sb.append(t)

        rpool = es.enter_context(tc.tile_pool(name="res", bufs=1))
        qT = [rpool.tile([128, S], F16, name=f"qT{i}", tag=f"qT{i}")
              for i in range(HPC)]
        kT = [rpool.tile([128, S], F16, name=f"kT{i}", tag=f"kT{i}")
              for i in range(HPC)]
        vsb = [rpool.tile([128, HJ], F16, name=f"v{j}", tag=f"v{j}")
               for j in range(NS)]

        # ---------------- phase 1: q/k/v projections ----------------------
        # qT[h]/kT[h] = W_h @ x^T via lhsT = W^T tiles (stationary) over
        # contraction d, rhs = x^T column groups. v in natural [s, hj] layout
        # via stationary x^T slices, moving Wv^T. Weight DMAs are batched as
        # [128, 512] tiles covering all 4 heads, loaded lazily inside the
        # first column group so the sync queue never starves the PE.
        with tc.tile_pool(name="wqk", bufs=1) as wpool, \
             tc.tile_pool(name="xt1", bufs=8) as xpool, \
             tc.tile_pool(name="wvp", bufs=1) as wvpool, \
             tc.tile_pool(name="xtv", bufs=8) as vxpool, \
             tc.tile_pool(name="ps1", bufs=1, space="PSUM") as ps1:
            wtile = {}
            wvt = {}
            for sg in range(NG):
                if sg == NG - 2:
                    # prefetch the v-projection weights while qk still computes
                    for d in range(ND):
                        t = wvpool.tile([128, HJ], F16, name=f"wv{d}",
                                        tag=f"wv{d}")
                        nc.sync.dma_start(t[:], wv[d * 128:(d + 1) * 128, :])
                        wvt[d] = t
                ps = {}
                for i in range(HPC):
                    ps[("q", i)] = ps1.tile([128, SG], F32, name=f"psa{i}",
                                            tag=f"a{i}")
                    ps[("k", i)] = ps1.tile([128, SG], F32, name=f"psb{i}",
                                            tag=f"b{i}")
                for d in range(ND):
                    xtile = xpool.tile([128, SG], F16, name="xtile", tag="xt")
                    nc.sync.dma_start(
                        xtile[:], xt[d * 128:(d + 1) * 128,
                                     sg * SG:(sg + 1) * SG])
                    for which, wdram in (("q", wq), ("k", wk)):
                        if (which, d) not in wtile:
                            t = wpool.tile([128, SG], F16, name=f"w{which}{d}",
                                           tag=f"w{which}{d}")
                            nc.sync.dma_start(
                                t[:], wdram[d * 128:(d + 1) * 128, :])
                            wtile[(which, d)] = t
                        for i in range(HPC):
                            nc.tensor.matmul(
                                ps[(which, i)][:],
                                lhsT=wtile[(which, d)][:, i * 128:(i + 1) * 128],
                                rhs=xtile[:],
                                start=(d == 0), stop=(d == ND - 1))
                for i in range(HPC):
                    nc.vector.tensor_scalar_add(
                        qT[i][:, sg * SG:(sg + 1) * SG], ps[("q", i)][:],
                        bq_sb[i][:])
                    nc.vector.tensor_scalar_add(
                        kT[i][:, sg * SG:(sg + 1) * SG], ps[("k", i)][:],
                        bk_sb[i][:])

            # v pass (re-streams x^T through its own pool; psum banks reuse
            # the q/k tags, alternating by sg parity for cross-sg overlap)
            for sg in range(NG):
                ab = "a" if sg % 2 == 0 else "b"
                ps = [ps1.tile([128, HJ], F32, name=f"psv{i}", tag=f"{ab}{i}")
                      for i in range(4)]
                for d in range(ND):
                    xtile = vxpool.tile([128, SG], F16, name="xtile", tag="xt")
                    nc.sync.dma_start(
                        xtile[:], xt[d * 128:(d + 1) * 128,
                                     sg * SG:(sg + 1) * SG])
                    for ss in range(4):
                        nc.tensor.matmul(
                            ps[ss][:],
                            lhsT=xtile[:, ss * 128:(ss + 1) * 128],
                            rhs=wvt[d][:],
                            start=(d == 0), stop=False)
                for ss in range(4):
                    # bias: rank-1 ones (x) bv accumulated into the same group
                    nc.tensor.matmul(
                        ps[ss][:], lhsT=ones_sb[:],
                        rhs=bv_sb[:], start=False, stop=True)
                    nc.vector.tensor_copy(vsb[sg * 4 + ss][:], ps[ss][:])

        # ---------------- phases 2+3: attention + output projection -------
        with tc.tile_pool(name="attn", bufs=1) as apool, \
             tc.tile_pool(name="wo", bufs=1) as wopool:
            attn = [apool.tile([128, S], F16, name=f"at{h}", tag=f"at{h}")
                    for h in range(HPC)]
            wot = []
            for t_ in range(HPC):
                wt = wopool.tile([128, D], F16, name=f"wo{t_}", tag=f"wo{t_}")
                nc.gpsimd.dma_start(wt[:], wo[t_ * 128:(t_ + 1) * 128, :])
                wot.append(wt)

            # phase 2: scores in [k, q] orientation; exp'd tiles feed PV as
            # the moving operand; sums via all-ones stationary (replicated
            # across partitions); normalize on the PSUM->SBUF copy. Units
            # ordered g-descending; phase 3 shares the pv psum slots and
            # backfills PE gaps as attn columns complete (st descending).
            with tc.tile_pool(name="et", bufs=8) as etpool, \
                 tc.tile_pool(name="sm", bufs=6) as spool, \
                 tc.tile_pool(name="ost", bufs=3) as opool, \
                 tc.tile_pool(name="ps_sc", bufs=3, space="PSUM") as ps_sc, \
                 tc.tile_pool(name="ps_x", bufs=2, space="PSUM") as ps_x, \
                 tc.tile_pool(name="ps_pv", bufs=3, space="PSUM") as ps_pv:
                for g in range(NG - 1, -1, -1):
                    nkt = 4 * g + 4
                    for h in range(HPC):
                        po = ps_pv.tile([128, SG], F32, name="popv", tag="pv")
                        sm = ps_x.tile([128, SG], F32, name="smps", tag="x")
                        for kt in range(nkt):
                            jlo = max(0, kt - 4 * g)
                            qoff = jlo * 128
                            w = SG - qoff
                            psc = ps_sc.tile([128, SG], F32, name="psc",
                                             tag="sc")
                            nc.tensor.matmul(
                                psc[:, :w],
                                lhsT=kT[h][:, kt * 128:(kt + 1) * 128],
                                rhs=qT[h][:, g * SG + qoff:(g + 1) * SG],
                                start=True, stop=True)
                            if kt >= 4 * g:
                                # diagonal block is this tile's first 128 cols
                                nc.vector.tensor_tensor(
                                    psc[:, 0:128], psc[:, 0:128],
                                    mask_sb[:], op=ADD)
                            et = etpool.tile([128, SG], F16, name="et",
                                             tag="et")
                            nc.scalar.activation(et[:, :w], psc[:, :w], EXP)
                            nc.tensor.matmul(
                                po[:, qoff:],
                                lhsT=vsb[kt][:, h * 128:(h + 1) * 128],
                                rhs=et[:, :w],
                                start=(kt == 0), stop=(kt == nkt - 1))
                            nc.tensor.matmul(
                                sm[:, qoff:],
                                lhsT=onesm_sb[:],
                                rhs=et[:, :w],
                                start=(kt == 0), stop=(kt == nkt - 1))
                        rr = spool.tile([128, SG], F32, name="rr", tag="rr")
                        nc.vector.reciprocal(rr[:], sm[:])
                        nc.vector.tensor_tensor(
                            attn[h][:, g * SG:(g + 1) * SG], po[:], rr[:],
                            op=MUL)

                    # phase 3 slice for this g level: output rows st = 4g..4g+3
                    for st in range(4 * g + 3, 4 * g - 1, -1):
                        for dg in range(NG):
                            po3 = ps_pv.tile([128, SG], F32, name="po3",
                                             tag="pv")
                            for h in range(HPC):
                                nc.tensor.matmul(
                                    po3[:],
                                    lhsT=attn[h][:, st * 128:(st + 1) * 128],
                                    rhs=wot[h][:, dg * SG:(dg + 1) * SG],
                                    start=(h == 0), stop=(h == HPC - 1))
                            ot = opool.tile([128, SG], F32, name="ost",
                                            tag="ost")
                            nc.vector.tensor_copy(ot[:], po3[:])
                            nc.gpsimd.dma_start(
                                out[st * 128:(st + 1) * 128,
                                    dg * SG:(dg + 1) * SG], ot[:])

    nc.finalize()
    return nc


_NC_CACHE = []


def kernel(hidden_states, Wq, bq, Wk, bk, Wv, bv, Wo, bo, **_unused):
    global last_exec_time_ns

    hidden_states = np.asarray(hidden_states, dtype=np.float32)
    Wq = np.asarray(Wq, dtype=np.float32)
    Wk = np.asarray(Wk, dtype=np.float32)
    Wv = np.asarray(Wv, dtype=np.float32)
    Wo = np.asarray(Wo, dtype=np.float32)
    bq = np.asarray(bq, dtype=np.float32)
    bk = np.asarray(bk, dtype=np.float32)
    bv = np.asarray(bv, dtype=np.float32)
    bo = np.asarray(bo, dtype=np.float32)

    if not _NC_CACHE:
        _NC_CACHE.append(_build())
    nc = _NC_CACHE[0]

    scale = 1.0 / math.sqrt(HD)
    q_idx = np.arange(128)[:, None]
    k_idx = np.arange(128)[None, :]
    # [k, q] orientation: keep k <= q
    mask = np.where(k_idx.T <= q_idx.T, 0.0, -50.0).astype(np.float32)
    ones = np.ones((1, 128), np.float16)

    xts = [np.ascontiguousarray(hidden_states[b].T).astype(np.float16)
           for b in range(B)]
    in_maps = []
    for c in range(N_CORES):
        b, hg = divmod(c, HPC)
        sl = slice(hg * HJ, (hg + 1) * HJ)
        in_maps.append({
            "xt": xts[b],
            "wq": np.ascontiguousarray((Wq[sl] * scale).T).astype(np.float16),
            "wk": np.ascontiguousarray(Wk[sl].T).astype(np.float16),
            "wv": np.ascontiguousarray(Wv[sl].T).astype(np.float16),
            "wo": np.ascontiguousarray(Wo[:, sl].T).astype(np.float16),
            "bq": (bq[sl] * scale).reshape(HJ, 1).copy(),
            "bk": bk[sl].reshape(HJ, 1).copy(),
            "bv": bv[sl].reshape(1, HJ).astype(np.float16),
            "mask": mask,
            "ones": ones,
        })

    res = run_bass_kernel_spmd(nc, in_maps, core_ids=list(range(N_CORES)))
    last_exec_time_ns = res.exec_time_ns

    outp = np.empty((B, S, D), np.float32)
    for b in range(B):
        acc = res.results[b * HPC]["out"].astype(np.float32)
        for c in range(b * HPC + 1, (b + 1) * HPC):
            acc = acc + res.results[c]["out"]
        outp[b] = acc + bo[None, :]
    return outp



# revision 4
# speedup vs baseline: 1.0113x; 1.0113x over previous
"""Multi-head causal self-attention (B=2, S=2048, D=2048, H=16) on 8 TRN2 cores.

Sharding: data parallel on batch (2) x tensor parallel on head groups (4 heads
per core). Each core computes QKV projections for its 512 q/k/v channels, the
causal attention for its 4 heads, and a partial output projection against its
512 columns of Wo. The host sums the 4 partials per batch and adds bo.

All matmul operands are fp16 (full PE rate, fast weight load, fp22 multiply
with fp32 accumulate); softmax statistics stay fp32. Scores are computed
directly in [k, q] orientation so the exp'd tiles feed the PV matmul as the
moving operand with no transposes; row sums come from an all-ones stationary
matmul (replicated across partitions) and normalization happens on the
PSUM->SBUF copy.
"""

import math
from contextlib import ExitStack

import numpy as np

import concourse.bass as bass
import concourse.tile as tile
from concourse import bacc, mybir
from concourse.bass_utils import run_bass_kernel_spmd

B, S, D, H, HD = 2, 2048, 2048, 16, 128
N_CORES = 8
HPC = 4          # heads per core
HJ = HPC * HD    # 512 projection channels per core
SG = 512         # column-group width for matmuls
ND = D // 128    # 16 contraction tiles over model dim
NS = S // 128    # 16 tiles over sequence
NG = S // SG     # 4 column groups over sequence

F32 = mybir.dt.float32
F16 = mybir.dt.float16
AX = mybir.AxisListType.X
ADD = mybir.AluOpType.add
MUL = mybir.AluOpType.mult
EXP = mybir.ActivationFunctionType.Exp

last_exec_time_ns = None
last_result = None


def _build():
    nc = bacc.Bacc("TRN2", target_bir_lowering=False, debug=False)

    xt = nc.dram_tensor("xt", [D, S], F16, kind="ExternalInput").ap()
    wq = nc.dram_tensor("wq", [D, HJ], F16, kind="ExternalInput").ap()
    wk = nc.dram_tensor("wk", [D, HJ], F16, kind="ExternalInput").ap()
    wv = nc.dram_tensor("wv", [D, HJ], F16, kind="ExternalInput").ap()
    wo = nc.dram_tensor("wo", [HJ, D], F16, kind="ExternalInput").ap()
    bq = nc.dram_tensor("bq", [HJ, 1], F32, kind="ExternalInput").ap()
    bk = nc.dram_tensor("bk", [HJ, 1], F32, kind="ExternalInput").ap()
    bv = nc.dram_tensor("bv", [1, HJ], F16, kind="ExternalInput").ap()
    mask = nc.dram_tensor("mask", [128, 128], F32, kind="ExternalInput").ap()
    ones = nc.dram_tensor("ones", [1, 128], F16, kind="ExternalInput").ap()
    out = nc.dram_tensor("out", [S, D], F32, kind="ExternalOutput").ap()

    with tile.TileContext(nc) as tc, ExitStack() as es:
        cpool = es.enter_context(tc.tile_pool(name="const", bufs=1))
        mask_sb = cpool.tile([128, 128], F32, name="mask", tag="mask")
        nc.sync.dma_start(mask_sb[:], mask[:])
        ones_sb = cpool.tile([1, 128], F16, name="ones", tag="ones")
        nc.sync.dma_start(ones_sb[:], ones[:])
        bv_sb = cpool.tile([1, HJ], F16, name="bv", tag="bv")
        nc.sync.dma_start(bv_sb[:], bv[:])
        onesm_sb = cpool.tile([128, 128], F16, name="onesm_sb", tag="onesm")
        nc.gpsimd.memset(onesm_sb[:], 1.0)
        bq_sb = []
        bk_sb = []
        for i in range(HPC):
            t = cpool.tile([128, 1], F32, name=f"bq{i}", tag=f"bq{i}")
            nc.sync.dma_start(t[:], bq[i * 128:(i + 1) * 128, :])
            bq_sb.append(t)
            t = cpool.tile([128, 1], F32, name=f"bk{i}", tag=f"bk{i}")
            nc.sync.dma_start(t[:], bk[i * 128:(i + 1) * 128, :])
            bk_sb.append(t)

        rpool = es.enter_context(tc.tile_pool(name="res", bufs=1))
        qT = [rpool.tile([128, S], F16, name=f"qT{i}", tag=f"qT{i}")
              for i in range(HPC)]
        kT = [rpool.tile([128, S], F16, name=f"kT{i}", tag=f"kT{i}")
              for i in range(HPC)]
        vsb = [rpool.tile([128, HJ], F16, name=f"v{j}", tag=f"v{j}")
               for j in range(NS)]

        # ---------------- phase 1: q/k/v projections ----------------------
        # qT[h]/kT[h] = W_h @ x^T via lhsT = W^T tiles (stationary) over
        # contraction d, rhs = x^T column groups. v in natural [s, hj] layout
        # via stationary x^T slices, moving Wv^T. Weight DMAs are batched as
        # [128, 512] tiles covering all 4 heads, loaded lazily inside the
        # first column group so the sync queue never starves the PE.
        with tc.tile_pool(name="wqk", bufs=1) as wpool, \
             tc.tile_pool(name="xt1", bufs=8) as xpool, \
             tc.tile_pool(name="wvp", bufs=1) as wvpool, \
             tc.tile_pool(name="xtv", bufs=8) as vxpool, \
             tc.tile_pool(name="ps1", bufs=1, space="PSUM") as ps1:
            wtile = {}
            wvt = {}
            for sg in range(NG):
                if sg == NG - 2:
                    # prefetch the v-projection weights while qk still computes
                    for d in range(ND):
                        t = wvpool.tile([128, HJ], F16, name=f"wv{d}",
                                        tag=f"wv{d}")
                        nc.sync.dma_start(t[:], wv[d * 128:(d + 1) * 128, :])
                        wvt[d] = t
                ps = {}
                for i in range(HPC):
                    ps[("q", i)] = ps1.tile([128, SG], F32, name=f"psa{i}",
                                            tag=f"a{i}")
                    ps[("k", i)] = ps1.tile([128, SG], F32, name=f"psb{i}",
                                            tag=f"b{i}")
                for d in range(ND):
                    xtile = xpool.tile([128, SG], F16, name="xtile", tag="xt")
                    nc.sync.dma_start(
                        xtile[:], xt[d * 128:(d + 1) * 128,
                                     sg * SG:(sg + 1) * SG])
                    for which, wdram in (("q", wq), ("k", wk)):
                        if (which, d) not in wtile:
                            t = wpool.tile([128, SG], F16, name=f"w{which}{d}",
                                           tag=f"w{which}{d}")
                            nc.sync.dma_start(
                                t[:], wdram[d * 128:(d + 1) * 128, :])
                            wtile[(which, d)] = t
                        for i in range(HPC):
                            nc.tensor.matmul(
                                ps[(which, i)][:],
                                lhsT=wtile[(which, d)][:, i * 128:(i + 1) * 128],
                                rhs=xtile[:],
                                start=(d == 0), stop=(d == ND - 1))
                for i in range(HPC):
                    nc.vector.tensor_scalar_add(
                        qT[i][:, sg * SG:(sg + 1) * SG], ps[("q", i)][:],
                        bq_sb[i][:])
                    nc.vector.tensor_scalar_add(
                        kT[i][:, sg * SG:(sg + 1) * SG], ps[("k", i)][:],
                        bk_sb[i][:])

            # v pass (re-streams x^T through its own pool; psum banks reuse
            # the q/k tags, alternating by sg parity for cross-sg overlap)
            for sg in range(NG):
                ab = "a" if sg % 2 == 0 else "b"
                ps = [ps1.tile([128, HJ], F32, name=f"psv{i}", tag=f"{ab}{i}")
                      for i in range(4)]
                for d in range(ND):
                    xtile = vxpool.tile([128, SG], F16, name="xtile", tag="xt")
                    nc.sync.dma_start(
                        xtile[:], xt[d * 128:(d + 1) * 128,
                                     sg * SG:(sg + 1) * SG])
                    for ss in range(4):
                        nc.tensor.matmul(
                            ps[ss][:],
                            lhsT=xtile[:, ss * 128:(ss + 1) * 128],
                            rhs=wvt[d][:],
                            start=(d == 0), stop=False)
                for ss in range(4):
                    # bias: rank-1 ones (x) bv accumulated into the same group
                    nc.tensor.matmul(
                        ps[ss][:], lhsT=ones_sb[:],
                        rhs=bv_sb[:], start=False, stop=True)
                    nc.vector.tensor_copy(vsb[sg * 4 + ss][:], ps[ss][:])

        # ---------------- phases 2+3: attention + output projection -------
        with tc.tile_pool(name="attn", bufs=1) as apool, \
             tc.tile_pool(name="wo", bufs=1) as wopool:
            attn = [apool.tile([128, S], F16, name=f"at{h}", tag=f"at{h}")
                    for h in range(HPC)]
            wot = []
            for t_ in range(HPC):
                wt = wopool.tile([128, D], F16, name=f"wo{t_}", tag=f"wo{t_}")
                nc.gpsimd.dma_start(wt[:], wo[t_ * 128:(t_ + 1) * 128, :])
                wot.append(wt)

            # phase 2: scores in [k, q] orientation; exp'd tiles feed PV as
            # the moving operand; sums via all-ones stationary (replicated
            # across partitions); normalize on the PSUM->SBUF copy. Units
            # ordered g-descending; phase 3 shares the pv psum slots and
            # backfills PE gaps as attn columns complete (st descending).
            with tc.tile_pool(name="et", bufs=8) as etpool, \
                 tc.tile_pool(name="sm", bufs=6) as spool, \
                 tc.tile_pool(name="ost", bufs=3) as opool, \
                 tc.tile_pool(name="ps_sc", bufs=3, space="PSUM") as ps_sc, \
                 tc.tile_pool(name="ps_x", bufs=2, space="PSUM") as ps_x, \
                 tc.tile_pool(name="ps_pv", bufs=3, space="PSUM") as ps_pv:
                for g in range(NG - 1, -1, -1):
                    nkt = 4 * g + 4
                    for h in range(HPC):
                        po = ps_pv.tile([128, SG], F32, name="popv", tag="pv")
                        sm = ps_x.tile([128, SG], F32, name="smps", tag="x")
                        for kt in range(nkt):
                            jlo = max(0, kt - 4 * g)
                            qoff = jlo * 128
                            w = SG - qoff
                            psc = ps_sc.tile([128, SG], F32, name="psc",
                                             tag="sc")
                            nc.tensor.matmul(
                                psc[:, :w],
                                lhsT=kT[h][:, kt * 128:(kt + 1) * 128],
                                rhs=qT[h][:, g * SG + qoff:(g + 1) * SG],
                                start=True, stop=True)
                            if kt >= 4 * g:
                                # diagonal block is this tile's first 128 cols
                                nc.vector.tensor_tensor(
                                    psc[:, 0:128], psc[:, 0:128],
                                    mask_sb[:], op=ADD)
                            et = etpool.tile([128, SG], F16, name="et",
                                             tag="et")
                            nc.scalar.activation(et[:, :w], psc[:, :w], EXP)
                            nc.tensor.matmul(
                                po[:, qoff:],
                                lhsT=vsb[kt][:, h * 128:(h + 1) * 128],
                                rhs=et[:, :w],
                                start=(kt == 0), stop=(kt == nkt - 1))
                            nc.tensor.matmul(
                                sm[:, qoff:],
                                lhsT=onesm_sb[:],
                                rhs=et[:, :w],
                                start=(kt == 0), stop=(kt == nkt - 1))
                        rr = spool.tile([128, SG], F32, name="rr", tag="rr")
                        nc.vector.reciprocal(rr[:], sm[:])
                        nc.vector.tensor_tensor(
                            attn[h][:, g * SG:(g + 1) * SG], po[:], rr[:],
                            op=MUL)

                    # phase 3 slice for this g level: output rows st = 4g..4g+3
                    for st in range(4 * g + 3, 4 * g - 1, -1):
                        for dg in range(NG):
                            po3 = ps_pv.tile([128, SG], F32, name="po3",
                                             tag="pv")
                            for h in range(HPC):
                                nc.tensor.matmul(
                                    po3[:],
                                    lhsT=attn[h][:, st * 128:(st + 1) * 128],
                                    rhs=wot[h][:, dg * SG:(dg + 1) * SG],
                                    start=(h == 0), stop=(h == HPC - 1))
                            ot = opool.tile([128, SG], F32, name="ost",
                                            tag="ost")
                            nc.vector.tensor_copy(ot[:], po3[:])
                            nc.gpsimd.dma_start(
                                out[st * 128:(st + 1) * 128,
                                    dg * SG:(dg + 1) * SG], ot[:])

    nc.finalize()
    return nc


_NC_CACHE = []


def kernel(hidden_states, Wq, bq, Wk, bk, Wv, bv, Wo, bo, **_unused):
    global last_exec_time_ns, last_result

    hidden_states = np.asarray(hidden_states, dtype=np.float32)
    Wq = np.asarray(Wq, dtype=np.float32)
    Wk = np.asarray(Wk, dtype=np.float32)
    Wv = np.asarray(Wv, dtype=np.float32)
    Wo = np.asarray(Wo, dtype=np.float32)
    bq = np.asarray(bq, dtype=np.float32)
    bk = np.asarray(bk, dtype=np.float32)
    bv = np.asarray(bv, dtype=np.float32)
    bo = np.asarray(bo, dtype=np.float32)

    if not _NC_CACHE:
        _NC_CACHE.append(_build())
    nc = _NC_CACHE[0]

    scale = 1.0 / math.sqrt(HD)
    q_idx = np.arange(128)[:, None]
    k_idx = np.arange(128)[None, :]
    # [k, q] orientation: keep k <= q
    mask = np.where(k_idx.T <= q_idx.T, 0.0, -50.0).astype(np.float32)
    ones = np.ones((1, 128), np.float16)

    xts = [np.ascontiguousarray(hidden_states[b].T).astype(np.float16)
           for b in range(B)]
    in_maps = []
    for c in range(N_CORES):
        b, hg = divmod(c, HPC)
        sl = slice(hg * HJ, (hg + 1) * HJ)
        in_maps.append({
            "xt": xts[b],
            "wq": np.ascontiguousarray((Wq[sl] * scale).T).astype(np.float16),
            "wk": np.ascontiguousarray(Wk[sl].T).astype(np.float16),
            "wv": np.ascontiguousarray(Wv[sl].T).astype(np.float16),
            "wo": np.ascontiguousarray(Wo[:, sl].T).astype(np.float16),
            "bq": (bq[sl] * scale).reshape(HJ, 1).copy(),
            "bk": bk[sl].reshape(HJ, 1).copy(),
            "bv": bv[sl].reshape(1, HJ).astype(np.float16),
            "mask": mask,
            "ones": ones,
        })

    res = run_bass_kernel_spmd(nc, in_maps, core_ids=list(range(N_CORES)))
    last_exec_time_ns = res.exec_time_ns
    last_result = res

    outp = np.empty((B, S, D), np.float32)
    for b in range(B):
        acc = res.results[b * HPC]["out"].astype(np.float32)
        for c in range(b * HPC + 1, (b + 1) * HPC):
            acc = acc + res.results[c]["out"]
        outp[b] = acc + bo[None, :]
    return outp



# revision 6
# speedup vs baseline: 1.1213x; 1.1088x over previous
"""Multi-head causal self-attention (B=2, S=2048, D=2048, H=16) on 8 TRN2 cores.

Sharding: data parallel on batch (2) x tensor parallel on head groups (4 heads
per core). Each core computes QKV projections for its 512 q/k/v channels, the
causal attention for its 4 heads, and a partial output projection against its
512 columns of Wo. The host sums the 4 partials per batch and adds the biases
(bo plus the host-folded bv @ Wo.T term; bv commutes through the row-stochastic
attention).

All matmul operands are fp16 (full PE rate); softmax statistics stay fp32.
Scores are computed in [k, q] orientation so the exp'd tiles feed the PV
matmul as the moving operand with no transposes; row sums come from an
all-ones stationary matmul and normalization happens on the PSUM->SBUF copy.

Scheduling: projections are computed per sequence column-group (x streamed
once, q/k/v per head accumulated 16-deep into a single PSUM bank each), and
all Act-independent PE work (projection passes for the next column group,
output-projection units for the previous query group) is interleaved at
matmul granularity into the attention rows so the scalar-engine exp latency
never stalls the PE.
"""

import math
from collections import deque
from contextlib import ExitStack

import numpy as np

import concourse.bass as bass
import concourse.tile as tile
from concourse import bacc, mybir
from concourse.bass_utils import run_bass_kernel_spmd

B, S, D, H, HD = 2, 2048, 2048, 16, 128
N_CORES = 8
HPC = 4          # heads per core
HJ = HPC * HD    # 512 projection channels per core
SG = 512         # column-group width for matmuls
ND = D // 128    # 16 contraction tiles over model dim
NS = S // 128    # 16 tiles over sequence
NG = S // SG     # 4 column groups over sequence

F32 = mybir.dt.float32
F16 = mybir.dt.float16
ADD = mybir.AluOpType.add
MUL = mybir.AluOpType.mult
EXP = mybir.ActivationFunctionType.Exp

last_exec_time_ns = None
last_result = None


def _build():
    nc = bacc.Bacc("TRN2", target_bir_lowering=False, debug=False)

    xt = nc.dram_tensor("xt", [D, S], F16, kind="ExternalInput").ap()
    wq = nc.dram_tensor("wq", [D, HJ], F16, kind="ExternalInput").ap()
    wk = nc.dram_tensor("wk", [D, HJ], F16, kind="ExternalInput").ap()
    wv = nc.dram_tensor("wv", [D, HJ], F16, kind="ExternalInput").ap()
    wo = nc.dram_tensor("wo", [HJ, D], F16, kind="ExternalInput").ap()
    bq = nc.dram_tensor("bq", [HJ, 1], F32, kind="ExternalInput").ap()
    bk = nc.dram_tensor("bk", [HJ, 1], F32, kind="ExternalInput").ap()
    mask = nc.dram_tensor("mask", [128, 128], F32, kind="ExternalInput").ap()
    out = nc.dram_tensor("out", [S, D], F16, kind="ExternalOutput").ap()

    with tile.TileContext(nc) as tc, ExitStack() as es:
        # ---------------- SBUF residents ------------------------------
        cpool = es.enter_context(tc.tile_pool(name="const", bufs=1))
        wpool = es.enter_context(tc.tile_pool(name="wts", bufs=1))
        xpool = es.enter_context(tc.tile_pool(name="xin", bufs=2))
        rpool = es.enter_context(tc.tile_pool(name="res", bufs=1))
        etp = es.enter_context(tc.tile_pool(name="et", bufs=6))
        spool = es.enter_context(tc.tile_pool(name="sm", bufs=4))
        opool = es.enter_context(tc.tile_pool(name="ost", bufs=4))
        ps_pr = es.enter_context(tc.tile_pool(name="ps_pr", bufs=2,
                                              space="PSUM"))
        ps_sc = es.enter_context(tc.tile_pool(name="ps_sc", bufs=2,
                                              space="PSUM"))
        ps_po = es.enter_context(tc.tile_pool(name="ps_po", bufs=1,
                                              space="PSUM"))
        ps_sm = es.enter_context(tc.tile_pool(name="ps_sm", bufs=1,
                                              space="PSUM"))
        ps_o3 = es.enter_context(tc.tile_pool(name="ps_o3", bufs=2,
                                              space="PSUM"))

        # x tiles: SP queue.  weights: Act queue (idle until attention).
        xsb = {}

        def x_dma(sg):
            for d in range(ND):
                t = xpool.tile([128, SG], F16, name=f"x{d}", tag=f"x{d}")
                nc.sync.dma_start(
                    t[:], xt[d * 128:(d + 1) * 128, sg * SG:(sg + 1) * SG])
                xsb[(sg, d)] = t

        x_dma(0)

        wsb = {}
        for which, wdram in (("q", wq), ("k", wk), ("v", wv)):
            for d in range(ND):
                t = wpool.tile([128, HJ], F16, name=f"w{which}{d}",
                               tag=f"w{which}{d}")
                nc.scalar.dma_start(t[:], wdram[d * 128:(d + 1) * 128, :])
                wsb[(which, d)] = t
        wot = []
        for h in range(HPC):
            t = wpool.tile([128, D], F16, name=f"wo{h}", tag=f"wo{h}")
            nc.scalar.dma_start(t[:], wo[h * 128:(h + 1) * 128, :])
            wot.append(t)

        mask_sb = cpool.tile([128, 128], F32, name="mask", tag="mask")
        nc.sync.dma_start(mask_sb[:], mask[:])
        bq_sb = []
        bk_sb = []
        for i in range(HPC):
            t = cpool.tile([128, 1], F32, name=f"bq{i}", tag=f"bq{i}")
            nc.sync.dma_start(t[:], bq[i * 128:(i + 1) * 128, :])
            bq_sb.append(t)
            t = cpool.tile([128, 1], F32, name=f"bk{i}", tag=f"bk{i}")
            nc.sync.dma_start(t[:], bk[i * 128:(i + 1) * 128, :])
            bk_sb.append(t)
        onesm_sb = cpool.tile([128, 128], F16, name="onesm", tag="onesm")
        nc.gpsimd.memset(onesm_sb[:], 1.0)

        x_dma(1)

        qT = [rpool.tile([128, S], F16, name=f"qT{i}", tag=f"qT{i}")
              for i in range(HPC)]
        kT = [rpool.tile([128, S], F16, name=f"kT{i}", tag=f"kT{i}")
              for i in range(HPC)]
        vsb = [rpool.tile([128, HJ], F16, name=f"v{j}", tag=f"v{j}")
               for j in range(NS)]
        attn = [rpool.tile([128, S], F16, name=f"at{h}", tag=f"at{h}")
                for h in range(HPC)]

        # ---------------- micro-op generators -------------------------
        def proj_pass(sg, which, i):
            """One projection pass: 16 accumulating matmuls + biased copy.
            q/k: stationary W^T slice (out [hd, s]); v: stationary x slice
            (out [s, hj])."""
            ps = ps_pr.tile([128, SG], F32, name="pp", tag="pp")
            for d in range(ND):
                if which == "v":
                    yield lambda d=d: nc.tensor.matmul(
                        ps[:], lhsT=xsb[(sg, d)][:, i * 128:(i + 1) * 128],
                        rhs=wsb[("v", d)][:],
                        start=(d == 0), stop=(d == ND - 1))
                else:
                    yield lambda d=d: nc.tensor.matmul(
                        ps[:], lhsT=wsb[(which, d)][:, i * 128:(i + 1) * 128],
                        rhs=xsb[(sg, d)][:],
                        start=(d == 0), stop=(d == ND - 1))
            if which == "q":
                yield lambda: nc.vector.tensor_scalar_add(
                    qT[i][:, sg * SG:(sg + 1) * SG], ps[:], bq_sb[i][:])
            elif which == "k":
                yield lambda: nc.vector.tensor_scalar_add(
                    kT[i][:, sg * SG:(sg + 1) * SG], ps[:], bk_sb[i][:])
            else:
                yield lambda: nc.vector.tensor_copy(
                    vsb[sg * 4 + i][:], ps[:])

        def proj_sg(sg):
            for which in ("q", "k", "v"):
                for i in range(HPC):
                    yield from proj_pass(sg, which, i)

        def ph3_unit(st, dg):
            po3 = ps_o3.tile([128, SG], F32, name="po3", tag="po3")
            for h in range(HPC):
                yield lambda h=h: nc.tensor.matmul(
                    po3[:], lhsT=attn[h][:, st * 128:(st + 1) * 128],
                    rhs=wot[h][:, dg * SG:(dg + 1) * SG],
                    start=(h == 0), stop=(h == HPC - 1))
            ot = opool.tile([128, SG], F16, name="ost", tag="ost")
            yield lambda: nc.vector.tensor_copy(ot[:], po3[:])
            yield lambda: nc.gpsimd.dma_start(
                out[st * 128:(st + 1) * 128, dg * SG:(dg + 1) * SG], ot[:])

        def ph3_group(gg):
            for st in range(4 * gg, 4 * gg + 4):
                for dg in range(NG):
                    yield from ph3_unit(st, dg)

        def chain(gens):
            for g_ in gens:
                yield from g_

        # ---------------- sg0 projections (pure PE ramp-up) -----------
        for op in proj_sg(0):
            op()

        # ---------------- blocks: attention rows + interleaved filler --
        for g in range(NG):
            fillers = []
            n_micro = 0
            if g < NG - 1:
                if g + 2 < NG:
                    x_dma(g + 2)
                fillers.append(proj_sg(g + 1))
                n_micro += 12 * (ND + 1)
            if g >= 1:
                fillers.append(ph3_group(g - 1))
                n_micro += 16 * (HPC + 2)
            filler = chain(fillers)
            n_units = HPC * (4 * g + 4)
            per_unit = max(1, -(-n_micro // n_units))

            def pump(n):
                for _ in range(n):
                    op = next(filler, None)
                    if op is None:
                        return
                    op()

            nkt = 4 * g + 4
            for h in range(HPC):
                po = ps_po.tile([128, SG], F32, name="po", tag="po")
                sm = ps_sm.tile([128, SG], F32, name="sm", tag="sm")
                pend = deque()

                def flush():
                    kt, qoff, w, et = pend.popleft()
                    nc.tensor.matmul(
                        po[:, qoff:], lhsT=vsb[kt][:, h * 128:(h + 1) * 128],
                        rhs=et[:, :w], start=(kt == 0), stop=(kt == nkt - 1))
                    nc.tensor.matmul(
                        sm[:, qoff:], lhsT=onesm_sb[:], rhs=et[:, :w],
                        start=(kt == 0), stop=(kt == nkt - 1))

                for kt in range(nkt):
                    qoff = max(0, kt - 4 * g) * 128
                    w = SG - qoff
                    psc = ps_sc.tile([128, SG], F32, name="psc", tag="sc")
                    nc.tensor.matmul(
                        psc[:, :w], lhsT=kT[h][:, kt * 128:(kt + 1) * 128],
                        rhs=qT[h][:, g * SG + qoff:(g + 1) * SG],
                        start=True, stop=True)
                    if kt >= 4 * g:
                        nc.vector.tensor_tensor(
                            psc[:, 0:128], psc[:, 0:128], mask_sb[:], op=ADD)
                    et = etp.tile([128, SG], F16, name="et", tag="et")
                    nc.scalar.activation(et[:, :w], psc[:, :w], EXP)
                    pend.append((kt, qoff, w, et))
                    if len(pend) > 2:
                        flush()
                    pump(per_unit)
                while pend:
                    flush()
                rr = spool.tile([128, SG], F32, name="rr", tag="rr")
                nc.vector.reciprocal(rr[:], sm[:])
                nc.vector.tensor_tensor(
                    attn[h][:, g * SG:(g + 1) * SG], po[:], rr[:], op=MUL)
            pump(1 << 30)

        # ---------------- epilogue: last output-projection group ------
        for op in ph3_group(NG - 1):
            op()

    nc.finalize()
    return nc


_NC_CACHE = []


def kernel(hidden_states, Wq, bq, Wk, bk, Wv, bv, Wo, bo, **_unused):
    global last_exec_time_ns, last_result

    hidden_states = np.asarray(hidden_states, dtype=np.float32)
    Wq = np.asarray(Wq, dtype=np.float32)
    Wk = np.asarray(Wk, dtype=np.float32)
    Wv = np.asarray(Wv, dtype=np.float32)
    Wo = np.asarray(Wo, dtype=np.float32)
    bq = np.asarray(bq, dtype=np.float32)
    bk = np.asarray(bk, dtype=np.float32)
    bv = np.asarray(bv, dtype=np.float32)
    bo = np.asarray(bo, dtype=np.float32)

    if not _NC_CACHE:
        _NC_CACHE.append(_build())
    nc = _NC_CACHE[0]

    scale = 1.0 / math.sqrt(HD)
    q_idx = np.arange(128)[:, None]
    k_idx = np.arange(128)[None, :]
    # [k, q] orientation: keep k <= q
    mask = np.where(k_idx.T <= q_idx.T, 0.0, -50.0).astype(np.float32)

    xts = [np.ascontiguousarray(hidden_states[b].T).astype(np.float16)
           for b in range(B)]
    in_maps = []
    for c in range(N_CORES):
        b, hg = divmod(c, HPC)
        sl = slice(hg * HJ, (hg + 1) * HJ)
        in_maps.append({
            "xt": xts[b],
            "wq": np.ascontiguousarray((Wq[sl] * scale).T).astype(np.float16),
            "wk": np.ascontiguousarray(Wk[sl].T).astype(np.float16),
            "wv": np.ascontiguousarray(Wv[sl].T).astype(np.float16),
            "wo": np.ascontiguousarray(Wo[:, sl].T).astype(np.float16),
            "bq": (bq[sl] * scale).reshape(HJ, 1).copy(),
            "bk": bk[sl].reshape(HJ, 1).copy(),
            "mask": mask,
        })

    res = run_bass_kernel_spmd(nc, in_maps, core_ids=list(range(N_CORES)))
    last_exec_time_ns = res.exec_time_ns
    last_result = res

    # bv commutes through the row-stochastic attention into a constant
    # bv @ Wo.T shift on the output; fold it into the host bias add.
    bias_full = bo + bv @ Wo.T
    outp = np.empty((B, S, D), np.float32)
    for b in range(B):
        acc = res.results[b * HPC]["out"].astype(np.float32)
        for c in range(b * HPC + 1, (b + 1) * HPC):
            acc = acc + res.results[c]["out"].astype(np.float32)
        outp[b] = acc + bias_full[None, :]
    return outp


# revision 12
# speedup vs baseline: 1.2492x; 1.1141x over previous
"""Multi-head causal self-attention (B=2, S=2048, D=2048, H=16) on 8 TRN2 cores.

Sharding: data parallel on batch (2) x tensor parallel on head groups (4 heads
per core). Each core computes QKV projections for its 512 q/k/v channels, the
causal attention for its 4 heads, and a partial output projection against its
512 columns of Wo. The host sums the 4 partials per batch and adds the biases
(bo plus the host-folded bv @ Wo.T term; bv commutes through the row-stochastic
attention).

All matmul operands are fp16 (full PE rate); softmax statistics stay fp32.
Scores are computed in [k, q] orientation so the exp'd tiles feed the PV
matmul as the moving operand with no transposes; row sums come from an
all-ones stationary matmul and normalization happens on the PSUM->SBUF copy.

Scheduling: projections are computed per sequence column-group (x streamed
once, q/k/v per head accumulated 16-deep into a single PSUM bank each), and
all Act-independent PE work (projection passes for the next column group,
output-projection units for the previous query group) is interleaved at
matmul granularity into the attention rows so the scalar-engine exp latency
never stalls the PE.
"""

import math
from collections import deque
from contextlib import ExitStack

import numpy as np

import concourse.bass as bass
import concourse.tile as tile
from concourse import bacc, mybir
from concourse.bass_utils import run_bass_kernel_spmd

B, S, D, H, HD = 2, 2048, 2048, 16, 128
N_CORES = 8
HPC = 4          # heads per core
HJ = HPC * HD    # 512 projection channels per core
SG = 512         # column-group width for matmuls
ND = D // 128    # 16 contraction tiles over model dim
NS = S // 128    # 16 tiles over sequence
NG = S // SG     # 4 column groups over sequence

F32 = mybir.dt.float32
F16 = mybir.dt.float16
ADD = mybir.AluOpType.add
MUL = mybir.AluOpType.mult
EXP = mybir.ActivationFunctionType.Exp

last_exec_time_ns = None
last_result = None


def _build():
    nc = bacc.Bacc("TRN2", target_bir_lowering=False, debug=False)

    # x / w are host-repacked to [128, ...] with 4KB-per-partition
    # contiguous chunks so each DMA moves 512KB in one descriptor/partition.
    xt = nc.dram_tensor("xt", [128, NG * ND * SG], F16,
                        kind="ExternalInput").ap()
    wq = nc.dram_tensor("wq", [128, ND * SG], F16, kind="ExternalInput").ap()
    wk = nc.dram_tensor("wk", [128, ND * SG], F16, kind="ExternalInput").ap()
    wv = nc.dram_tensor("wv", [128, ND * SG], F16, kind="ExternalInput").ap()
    wo = nc.dram_tensor("wo", [HJ, D], F16, kind="ExternalInput").ap()
    bq = nc.dram_tensor("bq", [HJ, 1], F32, kind="ExternalInput").ap()
    bk = nc.dram_tensor("bk", [HJ, 1], F32, kind="ExternalInput").ap()
    mask = nc.dram_tensor("mask", [128, 128], F32, kind="ExternalInput").ap()
    out = nc.dram_tensor("out", [S, D], F16, kind="ExternalOutput").ap()

    with tile.TileContext(nc) as tc, ExitStack() as es:
        # ---------------- SBUF residents ------------------------------
        cpool = es.enter_context(tc.tile_pool(name="const", bufs=1))
        wpool = es.enter_context(tc.tile_pool(name="wts", bufs=1))
        xpool = es.enter_context(tc.tile_pool(name="xin", bufs=2))
        rpool = es.enter_context(tc.tile_pool(name="res", bufs=1))
        etp = es.enter_context(tc.tile_pool(name="et", bufs=6))
        spool = es.enter_context(tc.tile_pool(name="sm", bufs=4))
        opool = es.enter_context(tc.tile_pool(name="ost", bufs=4))
        ps_pr = es.enter_context(tc.tile_pool(name="ps_pr", bufs=2,
                                              space="PSUM"))
        ps_sc = es.enter_context(tc.tile_pool(name="ps_sc", bufs=2,
                                              space="PSUM"))
        ps_po = es.enter_context(tc.tile_pool(name="ps_po", bufs=1,
                                              space="PSUM"))
        ps_sm = es.enter_context(tc.tile_pool(name="ps_sm", bufs=1,
                                              space="PSUM"))
        ps_o3 = es.enter_context(tc.tile_pool(name="ps_o3", bufs=2,
                                              space="PSUM"))

        # x tiles: SP queue.  weights: Act queue (idle until attention).
        CW = 4 * SG          # 4 d-tiles per DMA chunk
        xsb = {}

        def x_dma(sg):
            for j in range(4):
                t = xpool.tile([128, CW], F16, name=f"x{j}", tag=f"x{j}")
                nc.sync.dma_start(
                    t[:], xt[:, sg * ND * SG + j * CW:
                             sg * ND * SG + (j + 1) * CW])
                xsb[(sg, j)] = t

        def xs(sg, d, lo, hi):
            return xsb[(sg, d // 4)][:, (d % 4) * SG + lo:(d % 4) * SG + hi]

        x_dma(0)

        wsb = {}
        for which, wdram in (("q", wq), ("k", wk), ("v", wv)):
            for j in range(4):
                t = wpool.tile([128, CW], F16, name=f"w{which}{j}",
                               tag=f"w{which}{j}")
                nc.scalar.dma_start(t[:], wdram[:, j * CW:(j + 1) * CW])
                wsb[(which, j)] = t

        def ws(which, d, lo, hi):
            return wsb[(which, d // 4)][:, (d % 4) * SG + lo:
                                        (d % 4) * SG + hi]
        wot = []
        for h in range(HPC):
            t = wpool.tile([128, D], F16, name=f"wo{h}", tag=f"wo{h}")
            nc.scalar.dma_start(t[:], wo[h * 128:(h + 1) * 128, :])
            wot.append(t)

        mask_sb = cpool.tile([128, 128], F32, name="mask", tag="mask")
        nc.sync.dma_start(mask_sb[:], mask[:])
        bq_sb = []
        bk_sb = []
        for i in range(HPC):
            t = cpool.tile([128, 1], F32, name=f"bq{i}", tag=f"bq{i}")
            nc.sync.dma_start(t[:], bq[i * 128:(i + 1) * 128, :])
            bq_sb.append(t)
            t = cpool.tile([128, 1], F32, name=f"bk{i}", tag=f"bk{i}")
            nc.sync.dma_start(t[:], bk[i * 128:(i + 1) * 128, :])
            bk_sb.append(t)
        onesm_sb = cpool.tile([128, 128], F16, name="onesm", tag="onesm")
        nc.gpsimd.memset(onesm_sb[:], 1.0)

        x_dma(1)

        qT = [rpool.tile([128, S], F16, name=f"qT{i}", tag=f"qT{i}")
              for i in range(HPC)]
        kT = [rpool.tile([128, S], F16, name=f"kT{i}", tag=f"kT{i}")
              for i in range(HPC)]
        vsb = [rpool.tile([128, HJ], F16, name=f"v{j}", tag=f"v{j}")
               for j in range(NS)]
        attn = [rpool.tile([128, S], F16, name=f"at{h}", tag=f"at{h}")
                for h in range(HPC)]

        # ---------------- micro-op generators -------------------------
        def proj_pass(sg, which, i):
            """One projection pass: 16 accumulating matmuls + biased copy.
            q/k: stationary W^T slice (out [hd, s]); v: stationary x slice
            (out [s, hj])."""
            ps = ps_pr.tile([128, SG], F32, name="pp", tag="pp")
            for d in range(ND):
                if which == "v":
                    yield lambda d=d: nc.tensor.matmul(
                        ps[:], lhsT=xs(sg, d, i * 128, (i + 1) * 128),
                        rhs=ws("v", d, 0, SG),
                        start=(d == 0), stop=(d == ND - 1))
                else:
                    yield lambda d=d: nc.tensor.matmul(
                        ps[:], lhsT=ws(which, d, i * 128, (i + 1) * 128),
                        rhs=xs(sg, d, 0, SG),
                        start=(d == 0), stop=(d == ND - 1))
            if which == "q":
                yield lambda: nc.vector.tensor_scalar_add(
                    qT[i][:, sg * SG:(sg + 1) * SG], ps[:], bq_sb[i][:])
            elif which == "k":
                yield lambda: nc.vector.tensor_scalar_add(
                    kT[i][:, sg * SG:(sg + 1) * SG], ps[:], bk_sb[i][:])
            else:
                yield lambda: nc.vector.tensor_copy(
                    vsb[sg * 4 + i][:], ps[:])

        def proj_sg(sg):
            for which in ("q", "k", "v"):
                for i in range(HPC):
                    yield from proj_pass(sg, which, i)

        def ph3_unit(st, dg):
            po3 = ps_o3.tile([128, SG], F32, name="po3", tag="po3")
            for h in range(HPC):
                yield lambda h=h: nc.tensor.matmul(
                    po3[:], lhsT=attn[h][:, st * 128:(st + 1) * 128],
                    rhs=wot[h][:, dg * SG:(dg + 1) * SG],
                    start=(h == 0), stop=(h == HPC - 1))
            ot = opool.tile([128, SG], F16, name="ost", tag="ost")
            yield lambda: nc.vector.tensor_copy(ot[:], po3[:])
            yield lambda: nc.sync.dma_start(
                out[st * 128:(st + 1) * 128, dg * SG:(dg + 1) * SG], ot[:])

        def ph3_group(gg):
            for st in range(4 * gg, 4 * gg + 4):
                for dg in range(NG):
                    yield from ph3_unit(st, dg)

        def chain(gens):
            for g_ in gens:
                yield from g_

        # ---------------- sg0 projections (pure PE ramp-up) -----------
        for op in proj_sg(0):
            op()

        # ---------------- blocks: attention rows + interleaved filler --
        for g in range(NG):
            fillers = []
            n_micro = 0
            if g < NG - 1:
                if g + 2 < NG:
                    x_dma(g + 2)
                fillers.append(proj_sg(g + 1))
                n_micro += 12 * (ND + 1)
            if g >= 1:
                fillers.append(ph3_group(g - 1))
                n_micro += 16 * (HPC + 2)
            filler = chain(fillers)
            n_units = HPC * (4 * g + 4)
            per_unit = max(1, -(-n_micro // n_units))

            def pump(n):
                for _ in range(n):
                    op = next(filler, None)
                    if op is None:
                        return
                    op()

            nkt = 4 * g + 4
            for h in range(HPC):
                po = ps_po.tile([128, SG], F32, name="po", tag="po")
                sm = ps_sm.tile([128, SG], F32, name="sm", tag="sm")
                pend = deque()

                def flush():
                    kt, qoff, w, et = pend.popleft()
                    nc.tensor.matmul(
                        po[:, qoff:], lhsT=vsb[kt][:, h * 128:(h + 1) * 128],
                        rhs=et[:, :w], start=(kt == 0), stop=(kt == nkt - 1))
                    nc.tensor.matmul(
                        sm[:, qoff:], lhsT=onesm_sb[:], rhs=et[:, :w],
                        start=(kt == 0), stop=(kt == nkt - 1))

                for kt in range(nkt):
                    qoff = max(0, kt - 4 * g) * 128
                    w = SG - qoff
                    psc = ps_sc.tile([128, SG], F32, name="psc", tag="sc")
                    nc.tensor.matmul(
                        psc[:, :w], lhsT=kT[h][:, kt * 128:(kt + 1) * 128],
                        rhs=qT[h][:, g * SG + qoff:(g + 1) * SG],
                        start=True, stop=True)
                    if kt >= 4 * g:
                        nc.vector.tensor_tensor(
                            psc[:, 0:128], psc[:, 0:128], mask_sb[:], op=ADD)
                    et = etp.tile([128, SG], F16, name="et", tag="et")
                    nc.scalar.activation(et[:, :w], psc[:, :w], EXP)
                    pend.append((kt, qoff, w, et))
                    if len(pend) > 2:
                        flush()
                    pump(per_unit)
                while pend:
                    flush()
                rr = spool.tile([128, SG], F32, name="rr", tag="rr")
                nc.vector.reciprocal_approx_fast(rr[:], sm[:])
                nc.vector.tensor_tensor(
                    attn[h][:, g * SG:(g + 1) * SG], po[:], rr[:], op=MUL)
            pump(1 << 30)

        # ---------------- epilogue: last output-projection group ------
        for op in ph3_group(NG - 1):
            op()

    nc.finalize()
    return nc


_NC_CACHE = []


def kernel(hidden_states, Wq, bq, Wk, bk, Wv, bv, Wo, bo, **_unused):
    global last_exec_time_ns, last_result

    hidden_states = np.asarray(hidden_states, dtype=np.float32)
    Wq = np.asarray(Wq, dtype=np.float32)
    Wk = np.asarray(Wk, dtype=np.float32)
    Wv = np.asarray(Wv, dtype=np.float32)
    Wo = np.asarray(Wo, dtype=np.float32)
    bq = np.asarray(bq, dtype=np.float32)
    bk = np.asarray(bk, dtype=np.float32)
    bv = np.asarray(bv, dtype=np.float32)
    bo = np.asarray(bo, dtype=np.float32)

    if not _NC_CACHE:
        _NC_CACHE.append(_build())
    nc = _NC_CACHE[0]

    scale = 1.0 / math.sqrt(HD)
    q_idx = np.arange(128)[:, None]
    k_idx = np.arange(128)[None, :]
    # [k, q] orientation: keep k <= q
    mask = np.where(k_idx.T <= q_idx.T, 0.0, -50.0).astype(np.float32)

    def pack_x(xt_ds):
        # [D, S] -> [128, sg, d, 512] with x[d*128+p, sg*512+c] at
        # [p, sg*8192 + d*512 + c]
        return np.ascontiguousarray(
            xt_ds.reshape(ND, 128, NG, SG).transpose(1, 2, 0, 3)
            .reshape(128, NG * ND * SG)).astype(np.float16)

    def pack_w(w_t):
        # [D, HJ] -> [128, d, 512] with w[d*128+p, c] at [p, d*512 + c]
        return np.ascontiguousarray(
            w_t.reshape(ND, 128, HJ).transpose(1, 0, 2)
            .reshape(128, ND * HJ)).astype(np.float16)

    xts = [pack_x(hidden_states[b].T) for b in range(B)]
    in_maps = []
    for c in range(N_CORES):
        b, hg = divmod(c, HPC)
        sl = slice(hg * HJ, (hg + 1) * HJ)
        in_maps.append({
            "xt": xts[b],
            "wq": pack_w((Wq[sl] * scale).T),
            "wk": pack_w(Wk[sl].T),
            "wv": pack_w(Wv[sl].T),
            "wo": np.ascontiguousarray(Wo[:, sl].T).astype(np.float16),
            "bq": (bq[sl] * scale).reshape(HJ, 1).copy(),
            "bk": bk[sl].reshape(HJ, 1).copy(),
            "mask": mask,
        })

    res = run_bass_kernel_spmd(nc, in_maps, core_ids=list(range(N_CORES)))
    last_exec_time_ns = res.exec_time_ns
    last_result = res

    # bv commutes through the row-stochastic attention into a constant
    # bv @ Wo.T shift on the output; fold it into the host bias add.
    bias_full = bo + bv @ Wo.T
    outp = np.empty((B, S, D), np.float32)
    for b in range(B):
        acc = res.results[b * HPC]["out"].astype(np.float32)
        for c in range(b * HPC + 1, (b + 1) * HPC):
            acc = acc + res.results[c]["out"].astype(np.float32)
        outp[b] = acc + bias_full[None, :]
    return outp


# revision 17
# speedup vs baseline: 1.2516x; 1.0019x over previous
"""Multi-head causal self-attention (B=2, S=2048, D=2048, H=16) on 8 TRN2 cores.

Sharding: data parallel on batch (2) x tensor parallel on head groups (4 heads
per core). Each core computes QKV projections for its 512 q/k/v channels, the
causal attention for its 4 heads, and a partial output projection against its
512 columns of Wo. The host sums the 4 partials per batch and adds the biases
(bo plus the host-folded bv @ Wo.T term; bv commutes through the row-stochastic
attention).

All matmul operands are fp16 (full PE rate); softmax statistics stay fp32.
Scores are computed in [k, q] orientation so the exp'd tiles feed the PV
matmul as the moving operand with no transposes; row sums come from an
all-ones stationary matmul and normalization happens on the PSUM->SBUF copy.

Scheduling: projections are computed per sequence column-group (x streamed
once, q/k/v per head accumulated 16-deep into a single PSUM bank each), and
all Act-independent PE work (projection passes for the next column group,
output-projection units for the previous query group) is interleaved at
matmul granularity into the attention rows so the scalar-engine exp latency
never stalls the PE.
"""

import math
from collections import deque
from contextlib import ExitStack

import numpy as np

import concourse.bass as bass
import concourse.tile as tile
from concourse import bacc, mybir
from concourse.bass_utils import run_bass_kernel_spmd

B, S, D, H, HD = 2, 2048, 2048, 16, 128
N_CORES = 8
HPC = 4          # heads per core
HJ = HPC * HD    # 512 projection channels per core
SG = 512         # column-group width for matmuls
ND = D // 128    # 16 contraction tiles over model dim
NS = S // 128    # 16 tiles over sequence
NG = S // SG     # 4 column groups over sequence

F32 = mybir.dt.float32
F16 = mybir.dt.float16
ADD = mybir.AluOpType.add
MUL = mybir.AluOpType.mult
EXP = mybir.ActivationFunctionType.Exp

last_exec_time_ns = None
last_result = None


def _build():
    nc = bacc.Bacc("TRN2", target_bir_lowering=False, debug=False)

    # x / w are host-repacked to [128, ...] with 4KB-per-partition
    # contiguous chunks so each DMA moves 512KB in one descriptor/partition.
    xt = nc.dram_tensor("xt", [128, NG * ND * SG], F16,
                        kind="ExternalInput").ap()
    wq = nc.dram_tensor("wq", [128, ND * SG], F16, kind="ExternalInput").ap()
    wk = nc.dram_tensor("wk", [128, ND * SG], F16, kind="ExternalInput").ap()
    wv = nc.dram_tensor("wv", [128, ND * SG], F16, kind="ExternalInput").ap()
    wo = nc.dram_tensor("wo", [HJ, D], F16, kind="ExternalInput").ap()
    bq = nc.dram_tensor("bq", [HJ, 1], F32, kind="ExternalInput").ap()
    bk = nc.dram_tensor("bk", [HJ, 1], F32, kind="ExternalInput").ap()
    mask = nc.dram_tensor("mask", [128, 128], F32, kind="ExternalInput").ap()
    out = nc.dram_tensor("out", [S, D], F16, kind="ExternalOutput").ap()

    with tile.TileContext(nc) as tc, ExitStack() as es:
        # ---------------- SBUF residents ------------------------------
        cpool = es.enter_context(tc.tile_pool(name="const", bufs=1))
        wpool = es.enter_context(tc.tile_pool(name="wts", bufs=1))
        xpool = es.enter_context(tc.tile_pool(name="xin", bufs=2))
        rpool = es.enter_context(tc.tile_pool(name="res", bufs=1))
        etp = es.enter_context(tc.tile_pool(name="et", bufs=6))
        spool = es.enter_context(tc.tile_pool(name="sm", bufs=4))
        opool = es.enter_context(tc.tile_pool(name="ost", bufs=4))
        ps_pr = es.enter_context(tc.tile_pool(name="ps_pr", bufs=2,
                                              space="PSUM"))
        ps_sc = es.enter_context(tc.tile_pool(name="ps_sc", bufs=2,
                                              space="PSUM"))
        ps_po = es.enter_context(tc.tile_pool(name="ps_po", bufs=1,
                                              space="PSUM"))
        ps_sm = es.enter_context(tc.tile_pool(name="ps_sm", bufs=1,
                                              space="PSUM"))
        ps_o3 = es.enter_context(tc.tile_pool(name="ps_o3", bufs=2,
                                              space="PSUM"))

        # x tiles: SP queue.  weights: Act queue (idle until attention).
        CW = 4 * SG          # 4 d-tiles per DMA chunk
        # piecewise first loads: small leading pieces land on separate HW
        # queues so the first matmul isn't gated on one 512KB transfer
        LEAD = [(0, 1), (1, 2), (2, 4), (4, 6), (6, 8), (8, 12), (12, 16)]
        xsb = {}

        def x_dma(sg, pieces=None):
            tiles = [xpool.tile([128, CW], F16, name=f"x{j}", tag=f"x{j}")
                     for j in range(4)]
            for j in range(4):
                xsb[(sg, j)] = tiles[j]
            if pieces is None:
                pieces = [(4 * j, 4 * j + 4) for j in range(4)]
            for lo, hi in pieces:
                t = tiles[lo // 4]
                nc.sync.dma_start(
                    t[:, (lo % 4) * SG:(lo % 4) * SG + (hi - lo) * SG],
                    xt[:, sg * ND * SG + lo * SG:sg * ND * SG + hi * SG])

        def xs(sg, d, lo, hi):
            return xsb[(sg, d // 4)][:, (d % 4) * SG + lo:(d % 4) * SG + hi]

        wsb = {}

        def w_dma(which, wdram, pieces):
            tiles = [wpool.tile([128, CW], F16, name=f"w{which}{j}",
                                tag=f"w{which}{j}") for j in range(4)]
            for j in range(4):
                wsb[(which, j)] = tiles[j]
            for lo, hi in pieces:
                t = tiles[lo // 4]
                nc.scalar.dma_start(
                    t[:, (lo % 4) * SG:(lo % 4) * SG + (hi - lo) * SG],
                    wdram[:, lo * SG:hi * SG])

        x_dma(0, LEAD)
        w_dma("q", wq, LEAD)

        def ws(which, d, lo, hi):
            return wsb[(which, d // 4)][:, (d % 4) * SG + lo:
                                        (d % 4) * SG + hi]

        for which, wdram in (("k", wk), ("v", wv)):
            w_dma(which, wdram, [(4 * j, 4 * j + 4) for j in range(4)])
        wot = []
        for h in range(HPC):
            t = wpool.tile([128, D], F16, name=f"wo{h}", tag=f"wo{h}")
            nc.scalar.dma_start(t[:], wo[h * 128:(h + 1) * 128, :])
            wot.append(t)

        mask_sb = cpool.tile([128, 128], F32, name="mask", tag="mask")
        nc.sync.dma_start(mask_sb[:], mask[:])
        bq_sb = []
        bk_sb = []
        for i in range(HPC):
            t = cpool.tile([128, 1], F32, name=f"bq{i}", tag=f"bq{i}")
            nc.sync.dma_start(t[:], bq[i * 128:(i + 1) * 128, :])
            bq_sb.append(t)
            t = cpool.tile([128, 1], F32, name=f"bk{i}", tag=f"bk{i}")
            nc.sync.dma_start(t[:], bk[i * 128:(i + 1) * 128, :])
            bk_sb.append(t)
        onesm_sb = cpool.tile([128, SG], F16, name="onesm", tag="onesm")
        nc.gpsimd.memset(onesm_sb[:], 1.0)

        # warm the PE p-state during the initial DMA window with matmuls on
        # the memset tile; results land in rotating score psum, never read
        for _ in range(22):
            wm = ps_sc.tile([128, SG], F32, name="warm", tag="sc")
            nc.tensor.matmul(wm[:], lhsT=onesm_sb[:, 0:128],
                             rhs=onesm_sb[:], start=True, stop=True)

        x_dma(1)

        qT = [rpool.tile([128, S], F16, name=f"qT{i}", tag=f"qT{i}")
              for i in range(HPC)]
        kT = [rpool.tile([128, S], F16, name=f"kT{i}", tag=f"kT{i}")
              for i in range(HPC)]
        vsb = [rpool.tile([128, HJ], F16, name=f"v{j}", tag=f"v{j}")
               for j in range(NS)]
        attn = [rpool.tile([128, S], F16, name=f"at{h}", tag=f"at{h}")
                for h in range(HPC)]

        # ---------------- micro-op generators -------------------------
        def proj_pass(sg, which, i):
            """One projection pass: 16 accumulating matmuls + biased copy.
            q/k: stationary W^T slice (out [hd, s]); v: stationary x slice
            (out [s, hj])."""
            ps = ps_pr.tile([128, SG], F32, name="pp", tag="pp")
            for d in range(ND):
                if which == "v":
                    yield lambda d=d: nc.tensor.matmul(
                        ps[:], lhsT=xs(sg, d, i * 128, (i + 1) * 128),
                        rhs=ws("v", d, 0, SG),
                        start=(d == 0), stop=(d == ND - 1))
                else:
                    yield lambda d=d: nc.tensor.matmul(
                        ps[:], lhsT=ws(which, d, i * 128, (i + 1) * 128),
                        rhs=xs(sg, d, 0, SG),
                        start=(d == 0), stop=(d == ND - 1))
            if which == "q":
                yield lambda: nc.vector.tensor_scalar_add(
                    qT[i][:, sg * SG:(sg + 1) * SG], ps[:], bq_sb[i][:])
            elif which == "k":
                yield lambda: nc.vector.tensor_scalar_add(
                    kT[i][:, sg * SG:(sg + 1) * SG], ps[:], bk_sb[i][:])
            else:
                yield lambda: nc.vector.tensor_copy(
                    vsb[sg * 4 + i][:], ps[:])

        def proj_sg(sg):
            for which in ("q", "k", "v"):
                for i in range(HPC):
                    yield from proj_pass(sg, which, i)

        def ph3_unit(st, dg):
            po3 = ps_o3.tile([128, SG], F32, name="po3", tag="po3")
            for h in range(HPC):
                yield lambda h=h: nc.tensor.matmul(
                    po3[:], lhsT=attn[h][:, st * 128:(st + 1) * 128],
                    rhs=wot[h][:, dg * SG:(dg + 1) * SG],
                    start=(h == 0), stop=(h == HPC - 1))
            ot = opool.tile([128, SG], F16, name="ost", tag="ost")
            yield lambda: nc.vector.tensor_copy(ot[:], po3[:])
            yield lambda: nc.sync.dma_start(
                out[st * 128:(st + 1) * 128, dg * SG:(dg + 1) * SG], ot[:])

        def ph3_group(gg):
            for st in range(4 * gg, 4 * gg + 4):
                for dg in range(NG):
                    yield from ph3_unit(st, dg)

        def chain(gens):
            for g_ in gens:
                yield from g_

        # ---------------- sg0 projections (pure PE ramp-up) -----------
        for op in proj_sg(0):
            op()

        # ---------------- blocks: attention rows + interleaved filler --
        for g in range(NG):
            fillers = []
            n_micro = 0
            if g < NG - 1:
                if g + 2 < NG:
                    x_dma(g + 2)
                fillers.append(proj_sg(g + 1))
                n_micro += 12 * (ND + 1)
            if g >= 1:
                fillers.append(ph3_group(g - 1))
                n_micro += 16 * (HPC + 2)
            filler = chain(fillers)
            n_units = HPC * (4 * g + 4)
            per_unit = max(1, n_micro // n_units)

            def pump(n):
                for _ in range(n):
                    op = next(filler, None)
                    if op is None:
                        return
                    op()

            nkt = 4 * g + 4
            for h in range(HPC):
                po = ps_po.tile([128, SG], F32, name="po", tag="po")
                sm = ps_sm.tile([128, SG], F32, name="sm", tag="sm")
                pend = deque()

                def flush():
                    kt, qoff, w, et = pend.popleft()
                    nc.tensor.matmul(
                        po[:, qoff:], lhsT=vsb[kt][:, h * 128:(h + 1) * 128],
                        rhs=et[:, :w], start=(kt == 0), stop=(kt == nkt - 1))
                    nc.tensor.matmul(
                        sm[:, qoff:], lhsT=onesm_sb[:, 0:128], rhs=et[:, :w],
                        start=(kt == 0), stop=(kt == nkt - 1))

                for kt in range(nkt):
                    qoff = max(0, kt - 4 * g) * 128
                    w = SG - qoff
                    psc = ps_sc.tile([128, SG], F32, name="psc", tag="sc")
                    nc.tensor.matmul(
                        psc[:, :w], lhsT=kT[h][:, kt * 128:(kt + 1) * 128],
                        rhs=qT[h][:, g * SG + qoff:(g + 1) * SG],
                        start=True, stop=True)
                    if kt >= 4 * g:
                        nc.vector.tensor_tensor(
                            psc[:, 0:128], psc[:, 0:128], mask_sb[:], op=ADD)
                    et = etp.tile([128, SG], F16, name="et", tag="et")
                    nc.scalar.activation(et[:, :w], psc[:, :w], EXP)
                    pend.append((kt, qoff, w, et))
                    if len(pend) > 2:
                        flush()
                    pump(per_unit)
                while pend:
                    flush()
                rr = spool.tile([128, SG], F32, name="rr", tag="rr")
                nc.vector.reciprocal_approx_fast(rr[:], sm[:])
                nc.vector.tensor_tensor(
                    attn[h][:, g * SG:(g + 1) * SG], po[:], rr[:], op=MUL)
            pump(1 << 30)

        # ---------------- epilogue: last output-projection group ------
        for op in ph3_group(NG - 1):
            op()

    nc.finalize()
    return nc


_NC_CACHE = []


def kernel(hidden_states, Wq, bq, Wk, bk, Wv, bv, Wo, bo, **_unused):
    global last_exec_time_ns, last_result

    hidden_states = np.asarray(hidden_states, dtype=np.float32)
    Wq = np.asarray(Wq, dtype=np.float32)
    Wk = np.asarray(Wk, dtype=np.float32)
    Wv = np.asarray(Wv, dtype=np.float32)
    Wo = np.asarray(Wo, dtype=np.float32)
    bq = np.asarray(bq, dtype=np.float32)
    bk = np.asarray(bk, dtype=np.float32)
    bv = np.asarray(bv, dtype=np.float32)
    bo = np.asarray(bo, dtype=np.float32)

    if not _NC_CACHE:
        _NC_CACHE.append(_build())
    nc = _NC_CACHE[0]

    scale = 1.0 / math.sqrt(HD)
    q_idx = np.arange(128)[:, None]
    k_idx = np.arange(128)[None, :]
    # [k, q] orientation: keep k <= q
    mask = np.where(k_idx.T <= q_idx.T, 0.0, -50.0).astype(np.float32)

    def pack_x(xt_ds):
        # [D, S] -> [128, sg, d, 512] with x[d*128+p, sg*512+c] at
        # [p, sg*8192 + d*512 + c]
        return np.ascontiguousarray(
            xt_ds.reshape(ND, 128, NG, SG).transpose(1, 2, 0, 3)
            .reshape(128, NG * ND * SG)).astype(np.float16)

    def pack_w(w_t):
        # [D, HJ] -> [128, d, 512] with w[d*128+p, c] at [p, d*512 + c]
        return np.ascontiguousarray(
            w_t.reshape(ND, 128, HJ).transpose(1, 0, 2)
            .reshape(128, ND * HJ)).astype(np.float16)

    xts = [pack_x(hidden_states[b].T) for b in range(B)]
    in_maps = []
    for c in range(N_CORES):
        b, hg = divmod(c, HPC)
        sl = slice(hg * HJ, (hg + 1) * HJ)
        in_maps.append({
            "xt": xts[b],
            "wq": pack_w((Wq[sl] * scale).T),
            "wk": pack_w(Wk[sl].T),
            "wv": pack_w(Wv[sl].T),
            "wo": np.ascontiguousarray(Wo[:, sl].T).astype(np.float16),
            "bq": (bq[sl] * scale).reshape(HJ, 1).copy(),
            "bk": bk[sl].reshape(HJ, 1).copy(),
            "mask": mask,
        })

    res = run_bass_kernel_spmd(nc, in_maps, core_ids=list(range(N_CORES)))
    last_exec_time_ns = res.exec_time_ns
    last_result = res

    # bv commutes through the row-stochastic attention into a constant
    # bv @ Wo.T shift on the output; fold it into the host bias add.
    bias_full = bo + bv @ Wo.T
    outp = np.empty((B, S, D), np.float32)
    for b in range(B):
        acc = res.results[b * HPC]["out"].astype(np.float32)
        for c in range(b * HPC + 1, (b + 1) * HPC):
            acc = acc + res.results[c]["out"].astype(np.float32)
        outp[b] = acc + bias_full[None, :]
    return outp


# revision 20
# speedup vs baseline: 1.2663x; 1.0118x over previous
"""Multi-head causal self-attention (B=2, S=2048, D=2048, H=16) on 8 TRN2 cores.

Sharding: data parallel on batch (2) x tensor parallel on head groups (4 heads
per core). Each core computes QKV projections for its 512 q/k/v channels, the
causal attention for its 4 heads, and a partial output projection against its
512 columns of Wo. The host sums the 4 partials per batch and adds the biases
(bo plus the host-folded bv @ Wo.T term; bv commutes through the row-stochastic
attention).

All matmul operands are fp16 (full PE rate); softmax statistics stay fp32.
Scores are computed in [k, q] orientation so the exp'd tiles feed the PV
matmul as the moving operand with no transposes; row sums come from an
all-ones stationary matmul and normalization happens on the PSUM->SBUF copy.

Scheduling: projections are computed per sequence column-group (x streamed
once, q/k/v per head accumulated 16-deep into a single PSUM bank each), and
all Act-independent PE work (projection passes for the next column group,
output-projection units for the previous query group) is interleaved at
matmul granularity into the attention rows so the scalar-engine exp latency
never stalls the PE.
"""

import math
from collections import deque
from contextlib import ExitStack

import numpy as np

import concourse.bass as bass
import concourse.tile as tile
from concourse import bacc, mybir
from concourse.bass_utils import run_bass_kernel_spmd

B, S, D, H, HD = 2, 2048, 2048, 16, 128
N_CORES = 8
HPC = 4          # heads per core
HJ = HPC * HD    # 512 projection channels per core
SG = 512         # column-group width for matmuls
ND = D // 128    # 16 contraction tiles over model dim
NS = S // 128    # 16 tiles over sequence
NG = S // SG     # 4 column groups over sequence

F32 = mybir.dt.float32
F16 = mybir.dt.float16
ADD = mybir.AluOpType.add
MUL = mybir.AluOpType.mult
EXP = mybir.ActivationFunctionType.Exp

last_exec_time_ns = None
last_result = None


def _build():
    nc = bacc.Bacc("TRN2", target_bir_lowering=False, debug=False)

    # x / w are host-repacked to [128, ...] with 4KB-per-partition
    # contiguous chunks so each DMA moves 512KB in one descriptor/partition.
    xt = nc.dram_tensor("xt", [128, NG * ND * SG], F16,
                        kind="ExternalInput").ap()
    wq = nc.dram_tensor("wq", [128, ND * SG], F16, kind="ExternalInput").ap()
    wk = nc.dram_tensor("wk", [128, ND * SG], F16, kind="ExternalInput").ap()
    wv = nc.dram_tensor("wv", [128, ND * SG], F16, kind="ExternalInput").ap()
    wo = nc.dram_tensor("wo", [HJ, D], F16, kind="ExternalInput").ap()
    bq = nc.dram_tensor("bq", [HJ, 1], F32, kind="ExternalInput").ap()
    bk = nc.dram_tensor("bk", [HJ, 1], F32, kind="ExternalInput").ap()
    mask = nc.dram_tensor("mask", [128, 128], F32, kind="ExternalInput").ap()
    out = nc.dram_tensor("out", [S, D], F16, kind="ExternalOutput").ap()

    with tile.TileContext(nc) as tc, ExitStack() as es:
        # ---------------- SBUF residents ------------------------------
        cpool = es.enter_context(tc.tile_pool(name="const", bufs=1))
        wpool = es.enter_context(tc.tile_pool(name="wts", bufs=1))
        xpool = es.enter_context(tc.tile_pool(name="xin", bufs=2))
        rpool = es.enter_context(tc.tile_pool(name="res", bufs=1))
        etp = es.enter_context(tc.tile_pool(name="et", bufs=6))
        spool = es.enter_context(tc.tile_pool(name="sm", bufs=4))
        opool = es.enter_context(tc.tile_pool(name="ost", bufs=4))
        # main PSUM pools are entered after the 4-bank front pool closes

        # x tiles: SP queue.  weights: Act queue (idle until attention).
        CW = 4 * SG          # 4 d-tiles per DMA chunk
        # piecewise first loads: small leading pieces land on separate HW
        # queues so the first matmul isn't gated on one 512KB transfer
        LEAD = [(0, 1), (1, 2), (2, 4), (4, 6), (6, 8), (8, 12), (12, 16)]
        xsb = {}

        def x_dma(sg, pieces=None):
            tiles = [xpool.tile([128, CW], F16, name=f"x{j}", tag=f"x{j}")
                     for j in range(4)]
            for j in range(4):
                xsb[(sg, j)] = tiles[j]
            if pieces is None:
                pieces = [(4 * j, 4 * j + 4) for j in range(4)]
            for lo, hi in pieces:
                t = tiles[lo // 4]
                nc.sync.dma_start(
                    t[:, (lo % 4) * SG:(lo % 4) * SG + (hi - lo) * SG],
                    xt[:, sg * ND * SG + lo * SG:sg * ND * SG + hi * SG])

        def xs(sg, d, lo, hi):
            return xsb[(sg, d // 4)][:, (d % 4) * SG + lo:(d % 4) * SG + hi]

        wsb = {}

        def w_dma(which, wdram, pieces):
            tiles = [wpool.tile([128, CW], F16, name=f"w{which}{j}",
                                tag=f"w{which}{j}") for j in range(4)]
            for j in range(4):
                wsb[(which, j)] = tiles[j]
            for lo, hi in pieces:
                t = tiles[lo // 4]
                nc.scalar.dma_start(
                    t[:, (lo % 4) * SG:(lo % 4) * SG + (hi - lo) * SG],
                    wdram[:, lo * SG:hi * SG])

        x_dma(0, LEAD)
        w_dma("q", wq, LEAD)

        def ws(which, d, lo, hi):
            return wsb[(which, d // 4)][:, (d % 4) * SG + lo:
                                        (d % 4) * SG + hi]

        for which, wdram in (("k", wk), ("v", wv)):
            w_dma(which, wdram, [(4 * j, 4 * j + 4) for j in range(4)])
        wot = []
        for h in range(HPC):
            t = wpool.tile([128, D], F16, name=f"wo{h}", tag=f"wo{h}")
            nc.scalar.dma_start(t[:], wo[h * 128:(h + 1) * 128, :])
            wot.append(t)

        mask_sb = cpool.tile([128, 128], F32, name="mask", tag="mask")
        nc.sync.dma_start(mask_sb[:], mask[:])
        bq_sb = []
        bk_sb = []
        for i in range(HPC):
            t = cpool.tile([128, 1], F32, name=f"bq{i}", tag=f"bq{i}")
            nc.sync.dma_start(t[:], bq[i * 128:(i + 1) * 128, :])
            bq_sb.append(t)
            t = cpool.tile([128, 1], F32, name=f"bk{i}", tag=f"bk{i}")
            nc.sync.dma_start(t[:], bk[i * 128:(i + 1) * 128, :])
            bk_sb.append(t)
        onesm_sb = cpool.tile([128, SG], F16, name="onesm", tag="onesm")
        nc.gpsimd.memset(onesm_sb[:], 1.0)

        x_dma(1)

        qT = [rpool.tile([128, S], F16, name=f"qT{i}", tag=f"qT{i}")
              for i in range(HPC)]
        kT = [rpool.tile([128, S], F16, name=f"kT{i}", tag=f"kT{i}")
              for i in range(HPC)]
        vsb = [rpool.tile([128, HJ], F16, name=f"v{j}", tag=f"v{j}")
               for j in range(NS)]
        attn = [rpool.tile([128, S], F16, name=f"at{h}", tag=f"at{h}")
                for h in range(HPC)]

        # ---------------- front: sg0 q/k, DMA-arrival-paced ------------
        # 4 heads accumulate d-interleaved in 4 banks so each arriving
        # (w, x) chunk feeds 4 matmuls; a few warmup matmuls on the memset
        # tile pre-ramp the PE p-state during the first transfers.
        with tc.tile_pool(name="ps_f", bufs=1, space="PSUM") as ps_f:
            for j in range(8):
                wm = ps_f.tile([128, SG], F32, name="warm", tag=f"pf{j % 4}")
                nc.tensor.matmul(wm[:], lhsT=onesm_sb[:, 0:128],
                                 rhs=onesm_sb[:], start=True, stop=True)
            for which, dst, bias in (("q", qT, bq_sb), ("k", kT, bk_sb)):
                ps4 = [ps_f.tile([128, SG], F32, name=f"pf{i}", tag=f"pf{i}")
                       for i in range(HPC)]
                for d in range(ND):
                    for i in range(HPC):
                        nc.tensor.matmul(
                            ps4[i][:], lhsT=ws(which, d, i * 128, (i + 1) * 128),
                            rhs=xs(0, d, 0, SG),
                            start=(d == 0), stop=(d == ND - 1))
                for i in range(HPC):
                    nc.vector.tensor_scalar_add(
                        dst[i][:, 0:SG], ps4[i][:], bias[i][:])

        ps_pr = es.enter_context(tc.tile_pool(name="ps_pr", bufs=2,
                                              space="PSUM"))
        ps_sc = es.enter_context(tc.tile_pool(name="ps_sc", bufs=2,
                                              space="PSUM"))
        ps_po = es.enter_context(tc.tile_pool(name="ps_po", bufs=1,
                                              space="PSUM"))
        ps_sm = es.enter_context(tc.tile_pool(name="ps_sm", bufs=1,
                                              space="PSUM"))
        ps_o3 = es.enter_context(tc.tile_pool(name="ps_o3", bufs=2,
                                              space="PSUM"))

        # ---------------- micro-op generators -------------------------
        def proj_pass(sg, which, i):
            """One projection pass: 16 accumulating matmuls + biased copy.
            q/k: stationary W^T slice (out [hd, s]); v: stationary x slice
            (out [s, hj])."""
            ps = ps_pr.tile([128, SG], F32, name="pp", tag="pp")
            for d in range(ND):
                if which == "v":
                    yield lambda d=d: nc.tensor.matmul(
                        ps[:], lhsT=xs(sg, d, i * 128, (i + 1) * 128),
                        rhs=ws("v", d, 0, SG),
                        start=(d == 0), stop=(d == ND - 1))
                else:
                    yield lambda d=d: nc.tensor.matmul(
                        ps[:], lhsT=ws(which, d, i * 128, (i + 1) * 128),
                        rhs=xs(sg, d, 0, SG),
                        start=(d == 0), stop=(d == ND - 1))
            if which == "q":
                yield lambda: nc.vector.tensor_scalar_add(
                    qT[i][:, sg * SG:(sg + 1) * SG], ps[:], bq_sb[i][:])
            elif which == "k":
                yield lambda: nc.vector.tensor_scalar_add(
                    kT[i][:, sg * SG:(sg + 1) * SG], ps[:], bk_sb[i][:])
            else:
                yield lambda: nc.vector.tensor_copy(
                    vsb[sg * 4 + i][:], ps[:])

        def proj_sg(sg):
            for which in ("q", "k", "v"):
                for i in range(HPC):
                    yield from proj_pass(sg, which, i)

        def ph3_unit(st, dg):
            po3 = ps_o3.tile([128, SG], F32, name="po3", tag="po3")
            for h in range(HPC):
                yield lambda h=h: nc.tensor.matmul(
                    po3[:], lhsT=attn[h][:, st * 128:(st + 1) * 128],
                    rhs=wot[h][:, dg * SG:(dg + 1) * SG],
                    start=(h == 0), stop=(h == HPC - 1))
            ot = opool.tile([128, SG], F16, name="ost", tag="ost")
            yield lambda: nc.vector.tensor_copy(ot[:], po3[:])
            yield lambda: nc.sync.dma_start(
                out[st * 128:(st + 1) * 128, dg * SG:(dg + 1) * SG], ot[:])

        def ph3_group(gg):
            for st in range(4 * gg, 4 * gg + 4):
                for dg in range(NG):
                    yield from ph3_unit(st, dg)

        def chain(gens):
            for g_ in gens:
                yield from g_

        # ---------------- sg0 v projections (q/k done in the front) ----
        for i in range(HPC):
            for op in proj_pass(0, "v", i):
                op()

        # ---------------- blocks: attention rows + interleaved filler --
        for g in range(NG):
            fillers = []
            n_micro = 0
            if g < NG - 1:
                if g + 2 < NG:
                    x_dma(g + 2)
                fillers.append(proj_sg(g + 1))
                n_micro += 12 * (ND + 1)
            if g >= 1:
                fillers.append(ph3_group(g - 1))
                n_micro += 16 * (HPC + 2)
            filler = chain(fillers)
            n_units = HPC * (4 * g + 4)
            per_unit = max(1, n_micro // n_units)

            def pump(n):
                for _ in range(n):
                    op = next(filler, None)
                    if op is None:
                        return
                    op()

            nkt = 4 * g + 4
            for h in range(HPC):
                po = ps_po.tile([128, SG], F32, name="po", tag="po")
                sm = ps_sm.tile([128, SG], F32, name="sm", tag="sm")
                pend = deque()

                def flush():
                    kt, qoff, w, et = pend.popleft()
                    nc.tensor.matmul(
                        po[:, qoff:], lhsT=vsb[kt][:, h * 128:(h + 1) * 128],
                        rhs=et[:, :w], start=(kt == 0), stop=(kt == nkt - 1))
                    nc.tensor.matmul(
                        sm[:, qoff:], lhsT=onesm_sb[:, 0:128], rhs=et[:, :w],
                        start=(kt == 0), stop=(kt == nkt - 1))

                for kt in range(nkt):
                    qoff = max(0, kt - 4 * g) * 128
                    w = SG - qoff
                    psc = ps_sc.tile([128, SG], F32, name="psc", tag="sc")
                    nc.tensor.matmul(
                        psc[:, :w], lhsT=kT[h][:, kt * 128:(kt + 1) * 128],
                        rhs=qT[h][:, g * SG + qoff:(g + 1) * SG],
                        start=True, stop=True)
                    if kt >= 4 * g:
                        nc.vector.tensor_tensor(
                            psc[:, 0:128], psc[:, 0:128], mask_sb[:], op=ADD)
                    et = etp.tile([128, SG], F16, name="et", tag="et")
                    nc.scalar.activation(et[:, :w], psc[:, :w], EXP)
                    pend.append((kt, qoff, w, et))
                    if len(pend) > 2:
                        flush()
                    pump(per_unit)
                while pend:
                    flush()
                rr = spool.tile([128, SG], F32, name="rr", tag="rr")
                nc.vector.reciprocal_approx_fast(rr[:], sm[:])
                nc.vector.tensor_tensor(
                    attn[h][:, g * SG:(g + 1) * SG], po[:], rr[:], op=MUL)
            pump(1 << 30)

        # ---------------- epilogue: last output-projection group ------
        for op in ph3_group(NG - 1):
            op()

    nc.finalize()
    return nc


_NC_CACHE = []


def kernel(hidden_states, Wq, bq, Wk, bk, Wv, bv, Wo, bo, **_unused):
    global last_exec_time_ns, last_result

    hidden_states = np.asarray(hidden_states, dtype=np.float32)
    Wq = np.asarray(Wq, dtype=np.float32)
    Wk = np.asarray(Wk, dtype=np.float32)
    Wv = np.asarray(Wv, dtype=np.float32)
    Wo = np.asarray(Wo, dtype=np.float32)
    bq = np.asarray(bq, dtype=np.float32)
    bk = np.asarray(bk, dtype=np.float32)
    bv = np.asarray(bv, dtype=np.float32)
    bo = np.asarray(bo, dtype=np.float32)

    if not _NC_CACHE:
        _NC_CACHE.append(_build())
    nc = _NC_CACHE[0]

    scale = 1.0 / math.sqrt(HD)
    q_idx = np.arange(128)[:, None]
    k_idx = np.arange(128)[None, :]
    # [k, q] orientation: keep k <= q
    mask = np.where(k_idx.T <= q_idx.T, 0.0, -50.0).astype(np.float32)

    def pack_x(xt_ds):
        # [D, S] -> [128, sg, d, 512] with x[d*128+p, sg*512+c] at
        # [p, sg*8192 + d*512 + c]
        return np.ascontiguousarray(
            xt_ds.reshape(ND, 128, NG, SG).transpose(1, 2, 0, 3)
            .reshape(128, NG * ND * SG)).astype(np.float16)

    def pack_w(w_t):
        # [D, HJ] -> [128, d, 512] with w[d*128+p, c] at [p, d*512 + c]
        return np.ascontiguousarray(
            w_t.reshape(ND, 128, HJ).transpose(1, 0, 2)
            .reshape(128, ND * HJ)).astype(np.float16)

    xts = [pack_x(hidden_states[b].T) for b in range(B)]
    in_maps = []
    for c in range(N_CORES):
        b, hg = divmod(c, HPC)
        sl = slice(hg * HJ, (hg + 1) * HJ)
        in_maps.append({
            "xt": xts[b],
            "wq": pack_w((Wq[sl] * scale).T),
            "wk": pack_w(Wk[sl].T),
            "wv": pack_w(Wv[sl].T),
            "wo": np.ascontiguousarray(Wo[:, sl].T).astype(np.float16),
            "bq": (bq[sl] * scale).reshape(HJ, 1).copy(),
            "bk": bk[sl].reshape(HJ, 1).copy(),
            "mask": mask,
        })

    res = run_bass_kernel_spmd(nc, in_maps, core_ids=list(range(N_CORES)))
    last_exec_time_ns = res.exec_time_ns
    last_result = res

    # bv commutes through the row-stochastic attention into a constant
    # bv @ Wo.T shift on the output; fold it into the host bias add.
    bias_full = bo + bv @ Wo.T
    outp = np.empty((B, S, D), np.float32)
    for b in range(B):
        acc = res.results[b * HPC]["out"].astype(np.float32)
        for c in range(b * HPC + 1, (b + 1) * HPC):
            acc = acc + res.results[c]["out"].astype(np.float32)
        outp[b] = acc + bias_full[None, :]
    return outp
